# revision 16
# baseline (speedup 1.0000x reference)
"""Symmetric-halved Euclidean distance matrix on 8 Trainium2 NeuronCores.

Decomposition: 16 column strips of 512. Core c owns strips 2c, 2c+1 and
computes, for each owned strip s, the blocks d(rows strip (s+d) mod 16,
cols strip s) for diagonal offsets d = 0..8. Every unordered strip pair
{u, v} is covered; the host mirrors each [512, 512] block to its transposed
position, so only ~56% of the matrix is computed on device.

Device-side math: PSUM = gram via fp8e4 DoubleRow matmuls (K=256 per pass).
The elementwise PSUM->SBUF drain folds in the -2 gram scale and is split
between the Activation engine (rows 0..255 of each block, Copy with
scale=-2) and the DVE (rows 256..511, scalar_tensor_tensor that also adds
the broadcast column-norm tile). The host adds the row norms and takes the
sqrt.

Corner savings: for the two diagonal blocks (d=0) the row>=256, col<256
quadrant is the mirror of the row<256, col>=256 one, so the DVE half only
computes cols 256..511. The two offset-8 blocks go further: each {u, u+8}
pair is computed by two cores (c and c+4); cores 4-7 get their slot-9/10
strips rotated by 256 rows, and each core computes only the cols 0..255
half of rows 0..255 and the cols 256..511 half of rows 256..511 — the four
quarter pieces tile the block with zero redundancy. This trims 6144 of
73728 PE cycles.
"""
import sys

sys.path.insert(0, "/opt/trn_rl_repo")

import numpy as np
import ml_dtypes

N, D, NCORES = 8192, 512, 8
P = 128
KO = D // P          # 4 contraction blocks of 128
KB = 2               # DoubleRow: 2 matmuls of K=256 cover D=512
NSTRIP = 16          # global 512-wide column strips
SW = N // NSTRIP     # 512 strip width
HW_ = SW // 2        # 256 half strip width
NLOC = 10            # local strips per core (window 2c..2c+9)
NSLOT = 11           # + slot 10: strip 2c+8 again (rotated on cores 4-7)
ND = 9               # diagonal offsets 0..8 per owned strip
NBLK = 2 * ND        # 18 [512, 512] blocks per core

# Emission order of blocks: for rl in 0..9: (0, rl) if rl<=8; (1, rl-1) if rl>=1
BLOCKS = []
for _rl in range(NLOC):
    if _rl <= ND - 1:
        BLOCKS.append((0, _rl))
    if _rl >= 1:
        BLOCKS.append((1, _rl - 1))

# blocks whose DVE half only computes cols 256..511 (diag + offset-8)
CORNER_T = {t for t, (s, dd) in enumerate(BLOCKS) if dd in (0, ND - 1)}

TRACE = False
LAST_EXEC_NS = None
LAST_RESULTS = None

_nc_cache = None


def _build():
    global _nc_cache
    if _nc_cache is not None:
        return _nc_cache

    import concourse.tile as tile
    from concourse import bacc, mybir

    f32 = mybir.dt.float32
    f16 = mybir.dt.float16
    f8 = mybir.dt.float8e4
    AF = mybir.ActivationFunctionType
    Alu = mybir.AluOpType
    PM = mybir.MatmulPerfMode

    nc = bacc.Bacc("TRN2", target_bir_lowering=False)
    # [p][b, i, j] packing of X^T per strip: row v*128+p, k = b*256+i*128+p
    xstat_d = nc.declare_dram_parameter(
        "xstat", [NSLOT * P, KB * 2 * SW], f8, isOutput=False
    )
    # [p][s, qq, j] = ||x_{strip s, col j}||^2 (broadcast over p and qq)
    ct_d = nc.declare_dram_parameter("ctrep", [P, 2 * 2 * SW], f16, isOutput=False)
    # 18 groups of [512, 512] fp16, laid out [g][p][q][i] so each partition's
    # DMA run is one contiguous 4 KB line
    out_d = nc.declare_dram_parameter("out", [NBLK * P, KO * SW], f16, isOutput=True)

    with tile.TileContext(nc) as tc:
        with (
            tc.tile_pool(name="res", bufs=1) as res,
            tc.tile_pool(name="stg", bufs=6) as stg,
            tc.tile_pool(name="mmps", bufs=4, space="PSUM") as mmps,
        ):
            xst = [
                res.tile([P, KB, 2, SW], f8, tag=f"xst{v}", name=f"xst{v}")
                for v in range(NSLOT)
            ]
            ct = res.tile([P, 2, 2, SW], f16, tag="ct")

            xstat_v = xstat_d[:].rearrange(
                "(v p) (b i j) -> v p b i j", p=P, b=KB, i=2
            )
            # All strip loads on sync in consumption order; the column-norm
            # tile rides gpsimd so it never delays a strip.
            for v in range(NSLOT):
                nc.sync.dma_start(xst[v], xstat_v[v])
            nc.gpsimd.dma_start(
                ct, ct_d[:].rearrange("p (s qq j) -> p s qq j", s=2, qq=2)
            )

            out_v = out_d[:].rearrange("(g p) (q i) -> g p q i", p=P, q=KO)

            for t, (s, dd) in enumerate(BLOCKS):
                # block (0,8) reads the dedicated (possibly rotated) slot 10;
                # slot 8 stays natural for block (1,7)
                rl = 10 if (s == 0 and dd == ND - 1) else s + dd
                stage = stg.tile([P, KO, SW], f16, tag="stage", name=f"st{t}")
                corner = t in CORNER_T
                off8 = dd == ND - 1
                for h in range(2):  # half-blocks: q in {2h, 2h+1}
                    ps = mmps.tile([P, 2, SW], f32, tag="mm", name=f"mm{t}_{h}")
                    # column window: diag h1 and off8 h1 take the right half;
                    # off8 h0 takes only the left half (the partner core's
                    # rotated pieces cover the rest)
                    j0 = HW_ if (corner and h == 1) else 0
                    j1 = HW_ if (off8 and h == 0) else SW
                    for qq in range(2):
                        q = 2 * h + qq
                        for b in range(KB):
                            nc.tensor.matmul(
                                ps[:, qq, j0:j1],
                                xst[rl][:, b, :, q * P:(q + 1) * P],
                                xst[s][:, b, :, j0:j1],
                                start=(b == 0),
                                stop=(b == KB - 1),
                                perf_mode=PM.DoubleRow,
                            )
                    if h == 0:
                        # rows 0..255: downcast with the -2 gram scale folded in
                        nc.scalar.activation(
                            stage[:, 0:2, j0:j1], ps[:, :, j0:j1],
                            AF.Copy, scale=-2.0,
                        )
                    else:
                        # rows 256..511: DVE applies -2 and adds the column norms
                        nc.vector.scalar_tensor_tensor(
                            stage[:, 2:4, j0:j1],
                            ps[:, :, j0:j1],
                            -2.0,
                            ct[:, s, :, j0:j1],
                            Alu.mult,
                            Alu.add,
                        )
                g = s * ND + dd
                if t < 10:
                    # early blocks ride gpsimd (sync is still streaming inputs)
                    nc.gpsimd.dma_start(out_v[g], stage)
                else:
                    # late blocks: h0 on the scalar ring (idle, trigger issued
                    # right after the ACT drain); h1 on sync first, then on
                    # gpsimd once its early-block queue has drained, so no
                    # single ring backlogs the tail
                    nc.scalar.dma_start(out_v[g, :, 0:2], stage[:, 0:2])
                    eng = nc.sync if t < 14 else nc.gpsimd
                    eng.dma_start(out_v[g, :, 2:4], stage[:, 2:4])

    nc.compile()
    _nc_cache = nc
    return nc


def _pack_fp8(xt8):
    """[D, N] fp8 -> per-strip [P, KB*2*SW] with k = b*256 + i*128 + p."""
    a = xt8.reshape(KB, 2, P, N).transpose(2, 0, 1, 3)  # [P, b, i, N]
    return [
        np.ascontiguousarray(a[:, :, :, g * SW:(g + 1) * SW].reshape(P, KB * 2 * SW))
        for g in range(NSTRIP)
    ]


def kernel(embeddings):
    global LAST_EXEC_NS, LAST_RESULTS
    emb = np.ascontiguousarray(np.asarray(embeddings, dtype=np.float32))
    assert emb.shape == (N, D)
    xt = np.ascontiguousarray(emb.T)                      # [D, N]
    sq = np.einsum("ij,ij->i", emb, emb).astype(np.float32)  # exact norms [N]

    xt8 = xt.astype(ml_dtypes.float8_e4m3)
    stat8 = _pack_fp8(xt8)
    # per-strip rotated-by-256 variants for cores 4-7's slot 9/10 strips
    xr = xt8.reshape(D, NSTRIP, SW)
    xrot = np.ascontiguousarray(
        np.concatenate([xr[:, :, HW_:], xr[:, :, :HW_]], axis=2).reshape(D, N)
    )
    rot8 = _pack_fp8(xrot)

    in_maps = []
    for c in range(NCORES):
        strips = [(2 * c + k) % NSTRIP for k in range(NLOC)]
        packs = [stat8[g] for g in strips]
        if c >= 4:
            packs.append(rot8[strips[8]])   # slot 10: rotated strip 2c+8
            packs[9] = rot8[strips[9]]
        else:
            packs.append(stat8[strips[8]])  # slot 10: natural copy
        xstat = np.concatenate(packs, axis=0)
        sqs = np.stack(
            [sq[strips[0] * SW:(strips[0] + 1) * SW],
             sq[strips[1] * SW:(strips[1] + 1) * SW]]
        )  # [2, SW]
        ctrep = np.ascontiguousarray(
            np.broadcast_to(sqs[None, :, None, :], (P, 2, 2, SW)).reshape(
                P, 2 * 2 * SW
            )
        ).astype(np.float16)
        in_maps.append({"xstat": xstat, "ctrep": ctrep})

    nc = _build()
    from concourse.bass_utils import run_bass_kernel_spmd

    kwargs = {}
    if TRACE:
        kwargs["trace"] = True
    try:
        r = run_bass_kernel_spmd(
            nc, in_maps, core_ids=list(range(NCORES)), **kwargs
        )
    except Exception:  # noqa: BLE001
        # A previously-profiled NEFF can leave one-shot NRT state that fails
        # the next execution; the failed attempt clears it.
        r = run_bass_kernel_spmd(
            nc, in_maps, core_ids=list(range(NCORES)), **kwargs
        )
    LAST_EXEC_NS = r.exec_time_ns
    LAST_RESULTS = r

    full = np.empty((N, N), dtype=np.float32)
    for c in range(NCORES):
        raw = r.results[c]["out"]                     # [18*128, 4*512] fp16
        strips = [(2 * c + k) % NSTRIP for k in range(NLOC)]
        # [g][p][q][i] -> block row q*128+p: [g][q][p][i]
        a = (
            raw.reshape(NBLK, P, KO, SW)
            .transpose(0, 2, 1, 3)
            .reshape(NBLK * SW, SW)
            .astype(np.float32)
        )
        # row-norm term for every block row (device never adds it); cores
        # 4-7's offset-8 blocks have their rows rotated by 256
        sa_parts = []
        for s, dd in sorted(BLOCKS, key=lambda b: b[0] * ND + b[1]):
            sa = sq[strips[s + dd] * SW:(strips[s + dd] + 1) * SW]
            if c >= 4 and dd == ND - 1:
                sa = np.roll(sa, -HW_)
            sa_parts.append(sa)
        a += np.concatenate(sa_parts)[:, None]
        # ACT half-blocks (rows 0..255 of every block) miss the column norms
        for s, dd in BLOCKS:
            g = s * ND + dd
            a[g * SW:g * SW + HW_] += sq[strips[s] * SW:(strips[s] + 1) * SW][
                None, :
            ]
        np.maximum(a, 0.0, out=a)
        np.sqrt(a, out=a)
        for t, (s, dd) in enumerate(BLOCKS):
            g = s * ND + dd
            sg = strips[s]                    # global column strip
            rg = strips[s + dd]               # global row strip
            blk = a[g * SW:(g + 1) * SW]
            R = slice(rg * SW, (rg + 1) * SW)
            C = slice(sg * SW, (sg + 1) * SW)
            if t not in CORNER_T:
                full[R, C] = blk
                full[C, R] = blk.T
                continue
            h0 = blk[0:HW_]                   # valid: all cols
            quad = blk[HW_:SW, HW_:SW]        # valid: cols 256..511
            if dd == 0:
                # diagonal block: missing quadrant is the mirror of h0's right
                fb = np.empty((SW, SW), dtype=np.float32)
                fb[0:HW_] = h0
                fb[HW_:SW, HW_:SW] = quad
                fb[HW_:SW, 0:HW_] = h0[:, HW_:SW].T
                full[R, C] = fb
                full[C, R] = fb.T
                continue
            # offset-8 block: this core contributes two quarter pieces; the
            # complementary quarters come from core (c+4)%8's rotated block
            q00 = blk[0:HW_, 0:HW_]           # valid: rows 0..255, cols 0..255
            if c < 4:
                full[rg * SW:rg * SW + HW_, sg * SW:sg * SW + HW_] = q00
                full[sg * SW:sg * SW + HW_, rg * SW:rg * SW + HW_] = q00.T
                full[rg * SW + HW_:(rg + 1) * SW, sg * SW + HW_:(sg + 1) * SW] = quad
                full[sg * SW + HW_:(sg + 1) * SW, rg * SW + HW_:(rg + 1) * SW] = quad.T
            else:
                # rows are rotated: computed row r' is global row r'+256 mod 512
                full[rg * SW + HW_:(rg + 1) * SW, sg * SW:sg * SW + HW_] = q00
                full[sg * SW:sg * SW + HW_, rg * SW + HW_:(rg + 1) * SW] = q00.T
                full[rg * SW:rg * SW + HW_, sg * SW + HW_:(sg + 1) * SW] = quad
                full[sg * SW + HW_:(sg + 1) * SW, rg * SW:rg * SW + HW_] = quad.T
    np.fill_diagonal(full, 0.0)
    return full[None, :, :]


# revision 17
# speedup vs baseline: 1.1065x; 1.1065x over previous
"""Symmetric-halved Euclidean distance matrix on 8 Trainium2 NeuronCores.

Decomposition: 16 column strips of 512. Core c owns strips 2c, 2c+1 and
computes, for each owned strip s, the blocks d(rows strip (s+d) mod 16,
cols strip s) for diagonal offsets d = 0..8. Every unordered strip pair
{u, v} is covered; the host mirrors each [512, 512] block to its transposed
position, so only ~56% of the matrix is computed on device.

Device-side math: PSUM = gram via fp8e4 DoubleRow matmuls (K=256 per pass).
The elementwise PSUM->SBUF drain folds in the -2 gram scale and is split
between the Activation engine (rows 0..255 of each block, Copy with
scale=-2) and the DVE (rows 256..511, scalar_tensor_tensor that also adds
the broadcast column-norm tile). The host adds the row norms and takes the
sqrt.

Corner savings: for the two diagonal blocks (d=0) the row>=256, col<256
quadrant is the mirror of the row<256, col>=256 one, so the DVE half only
computes cols 256..511. The two offset-8 blocks go further: each {u, u+8}
pair is computed by two cores (c and c+4); cores 4-7 get their slot-9/10
strips rotated by 256 rows, and each core computes only the cols 0..255
half of rows 0..255 and the cols 256..511 half of rows 256..511 — the four
quarter pieces tile the block with zero redundancy. This trims 6144 of
73728 PE cycles.
"""
import sys

sys.path.insert(0, "/opt/trn_rl_repo")

import numpy as np
import ml_dtypes

N, D, NCORES = 8192, 512, 8
P = 128
KO = D // P          # 4 contraction blocks of 128
KB = 2               # DoubleRow: 2 matmuls of K=256 cover D=512
NSTRIP = 16          # global 512-wide column strips
SW = N // NSTRIP     # 512 strip width
HW_ = SW // 2        # 256 half strip width
NLOC = 10            # local strips per core (window 2c..2c+9)
NSLOT = 11           # + slot 10: strip 2c+8 again (rotated on cores 4-7)
ND = 9               # diagonal offsets 0..8 per owned strip
NBLK = 2 * ND        # 18 [512, 512] blocks per core

# Emission order of blocks: for rl in 0..9: (0, rl) if rl<=8; (1, rl-1) if rl>=1
BLOCKS = []
for _rl in range(NLOC):
    if _rl <= ND - 1:
        BLOCKS.append((0, _rl))
    if _rl >= 1:
        BLOCKS.append((1, _rl - 1))

# blocks whose DVE half only computes cols 256..511 (diag + offset-8)
CORNER_T = {t for t, (s, dd) in enumerate(BLOCKS) if dd in (0, ND - 1)}

TRACE = False
LAST_EXEC_NS = None
LAST_RESULTS = None

_nc_cache = None


def _build():
    global _nc_cache
    if _nc_cache is not None:
        return _nc_cache

    import concourse.tile as tile
    from concourse import bacc, mybir

    f32 = mybir.dt.float32
    f16 = mybir.dt.float16
    f8 = mybir.dt.float8e4
    AF = mybir.ActivationFunctionType
    Alu = mybir.AluOpType
    PM = mybir.MatmulPerfMode

    nc = bacc.Bacc("TRN2", target_bir_lowering=False)
    # [p][b, i, j] packing of X^T per strip: row v*128+p, k = b*256+i*128+p
    xstat_d = nc.declare_dram_parameter(
        "xstat", [NSLOT * P, KB * 2 * SW], f8, isOutput=False
    )
    # [p][s, qq, j] = ||x_{strip s, col j}||^2 (broadcast over p and qq)
    ct_d = nc.declare_dram_parameter("ctrep", [P, 2 * 2 * SW], f16, isOutput=False)
    # 18 groups of [512, 512] fp16, laid out [g][p][q][i] so each partition's
    # DMA run is one contiguous 4 KB line
    out_d = nc.declare_dram_parameter("out", [NBLK * P, KO * SW], f16, isOutput=True)

    with tile.TileContext(nc) as tc:
        with (
            tc.tile_pool(name="res", bufs=1) as res,
            tc.tile_pool(name="stg", bufs=6) as stg,
            tc.tile_pool(name="mmps", bufs=4, space="PSUM") as mmps,
        ):
            xst = [
                res.tile([P, KB, 2, SW], f8, tag=f"xst{v}", name=f"xst{v}")
                for v in range(NSLOT)
            ]
            ct = res.tile([P, 2, 2, SW], f16, tag="ct")

            xstat_v = xstat_d[:].rearrange(
                "(v p) (b i j) -> v p b i j", p=P, b=KB, i=2
            )
            # Strip loads interleaved across the sync and gpsimd rings in
            # consumption order (strip 0 first fleet-wide); the column-norm
            # tile rides gpsimd right after strip 1 so the first DVE drain
            # never waits on it.
            nc.sync.dma_start(xst[0], xstat_v[0])
            nc.gpsimd.dma_start(xst[1], xstat_v[1])
            nc.gpsimd.dma_start(
                ct, ct_d[:].rearrange("p (s qq j) -> p s qq j", s=2, qq=2)
            )
            for v in range(2, NSLOT):
                eng = nc.sync if v % 2 == 0 else nc.gpsimd
                eng.dma_start(xst[v], xstat_v[v])

            out_v = out_d[:].rearrange("(g p) (q i) -> g p q i", p=P, q=KO)

            for t, (s, dd) in enumerate(BLOCKS):
                # block (0,8) reads the dedicated (possibly rotated) slot 10;
                # slot 8 stays natural for block (1,7)
                rl = 10 if (s == 0 and dd == ND - 1) else s + dd
                stage = stg.tile([P, KO, SW], f16, tag="stage", name=f"st{t}")
                corner = t in CORNER_T
                off8 = dd == ND - 1
                for h in range(2):  # half-blocks: q in {2h, 2h+1}
                    ps = mmps.tile([P, 2, SW], f32, tag="mm", name=f"mm{t}_{h}")
                    # column window: diag h1 and off8 h1 take the right half;
                    # off8 h0 takes only the left half (the partner core's
                    # rotated pieces cover the rest)
                    j0 = HW_ if (corner and h == 1) else 0
                    j1 = HW_ if (off8 and h == 0) else SW
                    for qq in range(2):
                        q = 2 * h + qq
                        for b in range(KB):
                            nc.tensor.matmul(
                                ps[:, qq, j0:j1],
                                xst[rl][:, b, :, q * P:(q + 1) * P],
                                xst[s][:, b, :, j0:j1],
                                start=(b == 0),
                                stop=(b == KB - 1),
                                perf_mode=PM.DoubleRow,
                            )
                    if h == 0:
                        # rows 0..255: downcast with the -2 gram scale folded in
                        nc.scalar.activation(
                            stage[:, 0:2, j0:j1], ps[:, :, j0:j1],
                            AF.Copy, scale=-2.0,
                        )
                    else:
                        # rows 256..511: DVE applies -2 and adds the column norms
                        nc.vector.scalar_tensor_tensor(
                            stage[:, 2:4, j0:j1],
                            ps[:, :, j0:j1],
                            -2.0,
                            ct[:, s, :, j0:j1],
                            Alu.mult,
                            Alu.add,
                        )
                g = s * ND + dd
                if t < 10:
                    # early blocks ride gpsimd (sync is still streaming inputs)
                    nc.gpsimd.dma_start(out_v[g], stage)
                else:
                    # late blocks: split halves across the scalar ring (idle,
                    # trigger issued right after the ACT drain) and sync, so
                    # no single ring backlogs the tail
                    nc.scalar.dma_start(out_v[g, :, 0:2], stage[:, 0:2])
                    nc.sync.dma_start(out_v[g, :, 2:4], stage[:, 2:4])

    nc.compile()
    _nc_cache = nc
    return nc


def _pack_fp8(xt8):
    """[D, N] fp8 -> per-strip [P, KB*2*SW] with k = b*256 + i*128 + p."""
    a = xt8.reshape(KB, 2, P, N).transpose(2, 0, 1, 3)  # [P, b, i, N]
    return [
        np.ascontiguousarray(a[:, :, :, g * SW:(g + 1) * SW].reshape(P, KB * 2 * SW))
        for g in range(NSTRIP)
    ]


def kernel(embeddings):
    global LAST_EXEC_NS, LAST_RESULTS
    emb = np.ascontiguousarray(np.asarray(embeddings, dtype=np.float32))
    assert emb.shape == (N, D)
    xt = np.ascontiguousarray(emb.T)                      # [D, N]
    sq = np.einsum("ij,ij->i", emb, emb).astype(np.float32)  # exact norms [N]

    xt8 = xt.astype(ml_dtypes.float8_e4m3)
    stat8 = _pack_fp8(xt8)
    # per-strip rotated-by-256 variants for cores 4-7's slot 9/10 strips
    xr = xt8.reshape(D, NSTRIP, SW)
    xrot = np.ascontiguousarray(
        np.concatenate([xr[:, :, HW_:], xr[:, :, :HW_]], axis=2).reshape(D, N)
    )
    rot8 = _pack_fp8(xrot)

    in_maps = []
    for c in range(NCORES):
        strips = [(2 * c + k) % NSTRIP for k in range(NLOC)]
        packs = [stat8[g] for g in strips]
        if c >= 4:
            packs.append(rot8[strips[8]])   # slot 10: rotated strip 2c+8
            packs[9] = rot8[strips[9]]
        else:
            packs.append(stat8[strips[8]])  # slot 10: natural copy
        xstat = np.concatenate(packs, axis=0)
        sqs = np.stack(
            [sq[strips[0] * SW:(strips[0] + 1) * SW],
             sq[strips[1] * SW:(strips[1] + 1) * SW]]
        )  # [2, SW]
        ctrep = np.ascontiguousarray(
            np.broadcast_to(sqs[None, :, None, :], (P, 2, 2, SW)).reshape(
                P, 2 * 2 * SW
            )
        ).astype(np.float16)
        in_maps.append({"xstat": xstat, "ctrep": ctrep})

    nc = _build()
    from concourse.bass_utils import run_bass_kernel_spmd

    kwargs = {}
    if TRACE:
        kwargs["trace"] = True
    try:
        r = run_bass_kernel_spmd(
            nc, in_maps, core_ids=list(range(NCORES)), **kwargs
        )
    except Exception:  # noqa: BLE001
        # A previously-profiled NEFF can leave one-shot NRT state that fails
        # the next execution; the failed attempt clears it.
        r = run_bass_kernel_spmd(
            nc, in_maps, core_ids=list(range(NCORES)), **kwargs
        )
    LAST_EXEC_NS = r.exec_time_ns
    LAST_RESULTS = r

    full = np.empty((N, N), dtype=np.float32)
    for c in range(NCORES):
        raw = r.results[c]["out"]                     # [18*128, 4*512] fp16
        strips = [(2 * c + k) % NSTRIP for k in range(NLOC)]
        # [g][p][q][i] -> block row q*128+p: [g][q][p][i]
        a = (
            raw.reshape(NBLK, P, KO, SW)
            .transpose(0, 2, 1, 3)
            .reshape(NBLK * SW, SW)
            .astype(np.float32)
        )
        # row-norm term for every block row (device never adds it); cores
        # 4-7's offset-8 blocks have their rows rotated by 256
        sa_parts = []
        for s, dd in sorted(BLOCKS, key=lambda b: b[0] * ND + b[1]):
            sa = sq[strips[s + dd] * SW:(strips[s + dd] + 1) * SW]
            if c >= 4 and dd == ND - 1:
                sa = np.roll(sa, -HW_)
            sa_parts.append(sa)
        a += np.concatenate(sa_parts)[:, None]
        # ACT half-blocks (rows 0..255 of every block) miss the column norms
        for s, dd in BLOCKS:
            g = s * ND + dd
            a[g * SW:g * SW + HW_] += sq[strips[s] * SW:(strips[s] + 1) * SW][
                None, :
            ]
        np.maximum(a, 0.0, out=a)
        np.sqrt(a, out=a)
        for t, (s, dd) in enumerate(BLOCKS):
            g = s * ND + dd
            sg = strips[s]                    # global column strip
            rg = strips[s + dd]               # global row strip
            blk = a[g * SW:(g + 1) * SW]
            R = slice(rg * SW, (rg + 1) * SW)
            C = slice(sg * SW, (sg + 1) * SW)
            if t not in CORNER_T:
                full[R, C] = blk
                full[C, R] = blk.T
                continue
            h0 = blk[0:HW_]                   # valid: all cols
            quad = blk[HW_:SW, HW_:SW]        # valid: cols 256..511
            if dd == 0:
                # diagonal block: missing quadrant is the mirror of h0's right
                fb = np.empty((SW, SW), dtype=np.float32)
                fb[0:HW_] = h0
                fb[HW_:SW, HW_:SW] = quad
                fb[HW_:SW, 0:HW_] = h0[:, HW_:SW].T
                full[R, C] = fb
                full[C, R] = fb.T
                continue
            # offset-8 block: this core contributes two quarter pieces; the
            # complementary quarters come from core (c+4)%8's rotated block
            q00 = blk[0:HW_, 0:HW_]           # valid: rows 0..255, cols 0..255
            if c < 4:
                full[rg * SW:rg * SW + HW_, sg * SW:sg * SW + HW_] = q00
                full[sg * SW:sg * SW + HW_, rg * SW:rg * SW + HW_] = q00.T
                full[rg * SW + HW_:(rg + 1) * SW, sg * SW + HW_:(sg + 1) * SW] = quad
                full[sg * SW + HW_:(sg + 1) * SW, rg * SW + HW_:(rg + 1) * SW] = quad.T
            else:
                # rows are rotated: computed row r' is global row r'+256 mod 512
                full[rg * SW + HW_:(rg + 1) * SW, sg * SW:sg * SW + HW_] = q00
                full[sg * SW:sg * SW + HW_, rg * SW + HW_:(rg + 1) * SW] = q00.T
                full[rg * SW:rg * SW + HW_, sg * SW + HW_:(sg + 1) * SW] = quad
                full[sg * SW + HW_:(sg + 1) * SW, rg * SW:rg * SW + HW_] = quad.T
    np.fill_diagonal(full, 0.0)
    return full[None, :, :]
